# revision 1
# baseline (speedup 1.0000x reference)
"""Causal self-attention (RoPE, 16 heads) Trainium2 Bass kernel.

Problem: B=8, S=1024, D=1024, H=16, HS=64, fp32, causal + all-ones padding mask.

Strategy: data-parallel over batch — one batch element per NeuronCore (8 cores).
Per-core computation uses a fully "transposed activation" layout so no on-chip
transposes are needed beyond the initial x -> x^T:

  x^T   [D, S]   via 64 PE transposes of 128x128 tiles
  Q^T,K^T [D, S] = W^T @ x^T  (lhsT = W as stored, rhs = x^T)  + RoPE fused via
                   host-precomputed coefficient tiles (deinterleaved head layout
                   by permuting W_q/W_k columns; Q additionally scaled 1/sqrt(hs))
  V     [S, D]   = x @ W_v (lhsT = x^T chunks, rhs = W_v), stored per-head with
                   an appended ones-column so att@v also yields softmax sums
  S^T   [k, q]   = (K^T)^T-chunks @ Q^T  (per head, causal blocks only)
  att^T          = exp(S^T) (no max-subtraction needed: |scores| is small),
                   diag blocks masked by a host 0/1 triangle tile
  y^T   [D, S]   accumulated per head: lhsT = [v | 1] chunk, rhs = att^T chunk;
                   row 64 gives softmax sums; normalize with reciprocal
                   broadcast to 64 partitions via a DRAM-roundtrip DMA
  out   [S, D]   = y @ W_proj (lhsT = y^T chunks, rhs = W_proj)

All matmuls run in float32r (fp32 data, fast PE mode); everything else fp32.
"""

import os

# The Bass kernel executes through the axon PJRT backend and needs the
# NeuronCores visible; a JAX_PLATFORMS=cpu pin (used for jax reference
# computation) would hide them.
if "axon" not in os.environ.get("JAX_PLATFORMS", "axon"):
    os.environ.pop("JAX_PLATFORMS", None)

import numpy as np
from contextlib import ExitStack

import concourse.bass as bass
import concourse.mybir as mybir
import concourse.tile as tile
from concourse import bacc
from concourse.bass_utils import run_bass_kernel_spmd

B, S, D, H, HS = 8, 1024, 1024, 16, 64
P = 128
NCORES = 8
F32 = mybir.dt.float32
F32R = mybir.dt.float32r
EXP = mybir.ActivationFunctionType.Exp

_CACHE = {}


def _build_nc():
    nc = bacc.Bacc(
        "TRN2", target_bir_lowering=False, debug=False, num_devices=NCORES)
    x_d = nc.dram_tensor("x", [S, D], F32R, kind="ExternalInput")
    wq_d = nc.dram_tensor("wq", [D, D], F32R, kind="ExternalInput")
    wk_d = nc.dram_tensor("wk", [D, D], F32R, kind="ExternalInput")
    wv_d = nc.dram_tensor("wv", [D, D], F32R, kind="ExternalInput")
    wp_d = nc.dram_tensor("wp", [D, D], F32R, kind="ExternalInput")
    c1q_d = nc.dram_tensor("c1q", [P, S], F32, kind="ExternalInput")
    c2q_d = nc.dram_tensor("c2q", [P, S], F32, kind="ExternalInput")
    c1k_d = nc.dram_tensor("c1k", [P, S], F32, kind="ExternalInput")
    c2k_d = nc.dram_tensor("c2k", [P, S], F32, kind="ExternalInput")
    mask_d = nc.dram_tensor("mask", [P, P], F32, kind="ExternalInput")
    ident_d = nc.dram_tensor("ident", [P, P], F32R, kind="ExternalInput")
    ones_d = nc.dram_tensor("ones", [P, H], F32, kind="ExternalInput")
    zeros_d = nc.dram_tensor("zeros", [P, 384], F32, kind="ExternalInput")
    out_d = nc.dram_tensor("out", [S, D], F32, kind="ExternalOutput")

    def mm(out, lhsT, rhs, start, stop):
        nc.tensor.matmul(out, lhsT, rhs, start=start, stop=stop)

    with tile.TileContext(nc) as tc, ExitStack() as ctx:
        persist = ctx.enter_context(tc.tile_pool(name="persist", bufs=1))
        qt = [persist.tile([P, S], F32R, name=f"qt{i}", tag=f"qt{i}") for i in range(8)]
        kt = [persist.tile([P, S], F32R, name=f"kt{i}", tag=f"kt{i}") for i in range(8)]
        vt = [persist.tile([P, H, HS + 1], F32R, name=f"vt{i}", tag=f"vt{i}")
              for i in range(8)]
        c1q = persist.tile([P, S], F32, name="c1q_t", tag="c1q_t")
        c2q = persist.tile([P, S], F32, name="c2q_t", tag="c2q_t")
        c1k = persist.tile([P, S], F32, name="c1k_t", tag="c1k_t")
        c2k = persist.tile([P, S], F32, name="c2k_t", tag="c2k_t")
        maskt = persist.tile([P, P], F32, name="maskt", tag="maskt")
        for t, d_ in ((c1q, c1q_d), (c2q, c2q_d), (c1k, c1k_d), (c2k, c2k_d),
                      (maskt, mask_d)):
            nc.sync.dma_start(t[:], d_[:])
        ident = persist.tile([P, P], F32R, name="ident", tag="ident")
        nc.sync.dma_start(ident[:], ident_d[:])
        ones_t = persist.tile([P, H], F32, name="ones_t", tag="ones_t")
        nc.sync.dma_start(ones_t[:], ones_d[:])
        zeros_t = persist.tile([P, 384], F32, name="zeros_t", tag="zeros_t")
        nc.sync.dma_start(zeros_t[:], zeros_d[:])

        # ---------------- Phase A+B: x^T, QKV, RoPE ----------------
        with ExitStack() as pctx:
            xin = pctx.enter_context(tc.tile_pool(name="xin", bufs=3))
            xtp = pctx.enter_context(tc.tile_pool(name="xtp", bufs=1))
            xt = [xtp.tile([P, S], F32R, name=f"xt{i}", tag=f"xt{i}") for i in range(8)]
            wst = pctx.enter_context(tc.tile_pool(name="wst", bufs=18))
            wvst = pctx.enter_context(tc.tile_pool(name="wvst", bufs=9))
            rtmp = pctx.enter_context(tc.tile_pool(name="rtmp", bufs=3))
            pa = pctx.enter_context(tc.tile_pool(name="pa", bufs=3, space="PSUM"))
            pb = pctx.enter_context(tc.tile_pool(name="pb", bufs=4, space="PSUM"))

            for sc in range(8):
                xtile = xin.tile([P, D], F32R, name="xtile", tag="xin")
                nc.sync.dma_start(xtile[:], x_d[sc * P:(sc + 1) * P, :])
                for dc in range(8):
                    pt = pa.tile([P, P], F32, name="pt", tag="tp")
                    nc.tensor.matmul(
                        pt[:].bitcast(F32R),
                        xtile[:, dc * P:(dc + 1) * P],
                        ident[:],
                        is_transpose=True,
                    )
                    nc.vector.tensor_copy(xt[dc][:, sc * P:(sc + 1) * P], pt[:])

            def rope(ps, dst_slice, c1, c2, s0):
                # dst = ps * c1 + swap32(ps) * c2
                t = rtmp.tile([P, 512], F32, name="ropet", tag="rt")
                for g, src in ((0, 32), (1, 0), (2, 96), (3, 64)):
                    nc.scalar.copy(t[g * 32:(g + 1) * 32, :], ps[src:src + 32, :])
                nc.vector.tensor_mul(dst_slice, ps[:], c1[:, s0:s0 + 512])
                nc.vector.tensor_mul(t[:], t[:], c2[:, s0:s0 + 512])
                nc.vector.tensor_add(dst_slice, dst_slice, t[:])

            for wd, dst, c1, c2 in ((wq_d, qt, c1q, c2q), (wk_d, kt, c1k, c2k)):
                for fc in range(8):
                    wts = []
                    for dc in range(8):
                        wtile = wst.tile([P, P], F32R, name="wtile", tag="w")
                        nc.sync.dma_start(
                            wtile[:], wd[dc * P:(dc + 1) * P, fc * P:(fc + 1) * P])
                        wts.append(wtile)
                    for s2 in range(2):
                        ps = pb.tile([P, 512], F32, name="qkps", tag="qkps")
                        for dc in range(8):
                            mm(ps[:], wts[dc][:], xt[dc][:, s2 * 512:(s2 + 1) * 512],
                               dc == 0, dc == 7)
                        rope(ps, dst[fc][:, s2 * 512:(s2 + 1) * 512], c1, c2, s2 * 512)

            for f2 in range(2):
                wvts = []
                for dc in range(8):
                    wvtile = wvst.tile([P, 512], F32R, name="wvtile", tag="wv")
                    nc.sync.dma_start(
                        wvtile[:], wv_d[dc * P:(dc + 1) * P, f2 * 512:(f2 + 1) * 512])
                    wvts.append(wvtile)
                for sc in range(8):
                    ps = pb.tile([P, 512], F32, name="vps", tag="qkps")
                    for dc in range(8):
                        mm(ps[:], xt[dc][:, sc * P:(sc + 1) * P], wvts[dc][:],
                           dc == 0, dc == 7)
                    nc.vector.tensor_copy(
                        vt[sc][:, f2 * 8:(f2 + 1) * 8, 0:HS],
                        ps[:].rearrange("p (h e) -> p h e", e=HS))
            for sc in range(8):
                nc.vector.tensor_copy(vt[sc][:, :, HS], ones_t[:])

        # ---------------- Phase C+D ----------------
        with ExitStack() as cdctx:
            ytp = cdctx.enter_context(tc.tile_pool(name="ytp", bufs=1))
            yt = [ytp.tile([P, S], F32R, name=f"yt{i}", tag=f"yt{i}") for i in range(8)]

            with ExitStack() as cctx:
                attp = cctx.enter_context(tc.tile_pool(name="attp", bufs=17))
                smallp = cctx.enter_context(tc.tile_pool(name="smallp", bufs=4))
                pss_p = cctx.enter_context(tc.tile_pool(name="pss", bufs=5, space="PSUM"))
                psy_p = cctx.enter_context(tc.tile_pool(name="psy", bufs=3, space="PSUM"))

                def score_block(ft, hb, qc, kc):
                    # scores^T block then exp (only the causally allowed span)
                    pss = pss_p.tile([P, 512], F32, name="pss", tag="pss")
                    mm(pss[:], kt[ft][hb:hb + 64, kc * P:(kc + 1) * P],
                       qt[ft][hb:hb + 64, qc * 512:(qc + 1) * 512], True, True)
                    att = attp.tile([P, 512], F32R, name="att", tag="att")
                    qsub = kc * P - qc * 512
                    if 0 <= qsub < 512:
                        if qsub > 0:
                            nc.vector.tensor_copy(att[:, 0:qsub], zeros_t[:, 0:qsub])
                        nc.scalar.activation(att[:, qsub:], pss[:, qsub:], EXP)
                        nc.vector.tensor_mul(
                            att[:, qsub:qsub + P], att[:, qsub:qsub + P], maskt[:])
                    else:
                        nc.scalar.activation(att[:], pss[:], EXP)
                    return att

                for ft in range(8):
                    for qc in range(2):
                        kmax = 4 if qc == 0 else 8
                        psyA = psy_p.tile([HS + 1, 512], F32, name="psyA", tag="psy")
                        psyB = psy_p.tile([HS + 1, 512], F32, name="psyB", tag="psy")
                        # burst all score matmuls (adjacent K=64 pairs share the
                        # PE array via row groups 0/64); exps chase on ACT
                        atts = []
                        for kc in range(kmax):
                            atts.append(score_block(ft, 0, qc, kc))
                            atts.append(score_block(ft, 64, qc, kc))
                        for kc in range(kmax):
                            mm(psyA[:], vt[kc][:, 2 * ft, :], atts[2 * kc][:],
                               kc == 0, kc == kmax - 1)
                            mm(psyB[:], vt[kc][:, 2 * ft + 1, :], atts[2 * kc + 1][:],
                               kc == 0, kc == kmax - 1)
                        for hb, psy in ((0, psyA), (64, psyB)):
                            # free the psum bank ASAP (high-priority copies),
                            # then normalize off the PE critical path
                            srow = smallp.tile([1, 512], F32, name="srow",
                                               tag="srow")
                            with tc.high_priority(offset=200):
                                nc.vector.tensor_copy(
                                    yt[ft][hb:hb + 64, qc * 512:(qc + 1) * 512],
                                    psy[0:HS, :])
                                nc.vector.tensor_copy(srow[:], psy[HS:HS + 1, :])
                            rb = smallp.tile([P, 512], F32, name="rb", tag="rb")
                            nc.gpsimd.partition_broadcast(rb[:], srow[0:1, :])
                            nc.vector.reciprocal_approx_fast(out=rb[:], in_=rb[:])
                            sl = yt[ft][hb:hb + 64, qc * 512:(qc + 1) * 512]
                            nc.vector.tensor_mul(sl, sl, rb[hb:hb + 64, :])

            with ExitStack() as dctx:
                wpst = dctx.enter_context(tc.tile_pool(name="wpst", bufs=12))
                outp = dctx.enter_context(tc.tile_pool(name="outp", bufs=4))
                psp_p = dctx.enter_context(tc.tile_pool(name="psp", bufs=3, space="PSUM"))
                for n2 in range(2):
                    wpts = []
                    for dc in range(8):
                        wptile = wpst.tile([P, 512], F32R, name="wptile", tag="wp")
                        nc.sync.dma_start(
                            wptile[:], wp_d[dc * P:(dc + 1) * P, n2 * 512:(n2 + 1) * 512])
                        wpts.append(wptile)
                    for sc in range(8):
                        psp = psp_p.tile([P, 512], F32, name="psp", tag="psp")
                        for dc in range(8):
                            mm(psp[:], yt[dc][:, sc * P:(sc + 1) * P], wpts[dc][:],
                               dc == 0, dc == 7)
                        ot = outp.tile([P, 512], F32, name="ot", tag="ot")
                        nc.scalar.copy(ot[:], psp[:])
                        nc.sync.dma_start(
                            out_d[sc * P:(sc + 1) * P, n2 * 512:(n2 + 1) * 512], ot[:])
    nc.compile()
    return nc


def _prep(inputs):
    w_qkv = np.asarray(inputs["w_qkv"], np.float32)
    w_proj = np.asarray(inputs["w_proj"], np.float32)
    cos = np.asarray(inputs["cos"], np.float32).reshape(S, HS // 2)
    sin = np.asarray(inputs["sin"], np.float32).reshape(S, HS // 2)
    wq, wk, wv = w_qkv[:, 0:D], w_qkv[:, D:2 * D], w_qkv[:, 2 * D:3 * D]
    perm = np.empty(D, np.int64)
    for h in range(H):
        b0 = h * HS
        perm[b0:b0 + HS // 2] = b0 + np.arange(0, HS, 2)
        perm[b0 + HS // 2:b0 + HS] = b0 + np.arange(1, HS, 2)
    wq, wk = wq[:, perm], wk[:, perm]
    cosT = np.ascontiguousarray(cos.T)  # [32, S]
    sinT = np.ascontiguousarray(sin.T)
    c1 = np.concatenate([cosT, cosT, cosT, cosT], 0)  # [128, S]
    c2 = np.concatenate([-sinT, sinT, -sinT, sinT], 0)
    scale = np.float32(1.0 / np.sqrt(HS))
    mask = np.triu(np.ones((P, P), np.float32))  # [k, q]: allow q >= k
    common = {
        "wq": np.ascontiguousarray(wq), "wk": np.ascontiguousarray(wk),
        "wv": np.ascontiguousarray(wv), "wp": np.ascontiguousarray(w_proj),
        "c1q": c1 * scale, "c2q": c2 * scale, "c1k": c1, "c2k": c2,
        "mask": mask, "ident": np.eye(P, dtype=np.float32),
        "ones": np.ones((P, H), np.float32), "zeros": np.zeros((P, 384), np.float32),
    }
    return common


LAST_RESULT = None


def kernel(**inputs):
    global LAST_RESULT
    if "nc" not in _CACHE:
        _CACHE["nc"] = _build_nc()
    nc = _CACHE["nc"]
    common = _prep(inputs)
    x = np.asarray(inputs["x"], np.float32)
    in_maps = [dict(common, x=np.ascontiguousarray(x[b])) for b in range(B)]
    res = run_bass_kernel_spmd(nc, in_maps, list(range(NCORES)))
    LAST_RESULT = res
    out = np.stack([res.results[i]["out"] for i in range(B)], 0)
    return out.astype(np.float32)



# revision 14
# speedup vs baseline: 1.1828x; 1.1828x over previous
"""Causal self-attention (RoPE, 16 heads) Trainium2 Bass kernel.

Problem: B=8, S=1024, D=1024, H=16, HS=64, fp32 in/out, causal + all-ones mask.

Strategy: data-parallel over batch — one batch element per NeuronCore (8 cores).
All matmuls in bf16 (fp32 PSUM accumulation); fp32 only for PSUM, softmax
reciprocal, and the final output.

Per-core layout ("transposed activations", no on-chip transposes at all —
x^T is produced on the host):

  x^T   [D, S] bf16  host-transposed, DMA'd straight into SBUF
  Q^T,K^T [D, S]     = W^T @ x^T (lhsT = W tiles), RoPE applied via
                       deinterleaved-head column permutation of W (host) +
                       cos/sin coefficient tiles; swap-halves via DVE
                       partition-offset copies (off the ACT engine)
  V     [S, D]       = x @ W_v, stored per-head with an appended ones column
                       so att@v also yields the softmax denominators
  S^T   [k, q]       = K^T-chunks @ Q^T per head, causal blocks only; the
                       1/sqrt(hs) scale is folded into exp's scale immediate
  att   bf16         = exp(S^T) (no max subtraction: |scores| is small);
                       diagonal blocks masked by a 0/1 triangle multiply
  y^T   [D, S]       accumulated per head; row 64 = denominators; normalize
                       with reciprocal + gpsimd partition broadcast, fused
                       into the PSUM->SBUF move
  out   [S, D] fp32  = y @ W_proj

Emission order interleaves the attention of head-pair fc with the QKV
projection of fc+1 so the PE never idles long enough for the HAM clock gate
to re-throttle it; a burst of warmup matmuls at t=0 lifts the gate while the
initial DMAs stream in.
"""

import os

# The Bass kernel executes through the axon PJRT backend and needs the
# NeuronCores visible; a JAX_PLATFORMS=cpu pin (used for jax reference
# computation) would hide them.
if "axon" not in os.environ.get("JAX_PLATFORMS", "axon"):
    os.environ.pop("JAX_PLATFORMS", None)

import numpy as np
import ml_dtypes
from contextlib import ExitStack

import concourse.bass as bass
import concourse.mybir as mybir
import concourse.tile as tile
from concourse import bacc
from concourse.bass_utils import run_bass_kernel_spmd

B, S, D, H, HS = 8, 1024, 1024, 16, 64
P = 128
NCORES = 8
F32 = mybir.dt.float32
BF16 = mybir.dt.bfloat16
EXP = mybir.ActivationFunctionType.Exp
SCALE = 0.125  # 1/sqrt(HS)

_CACHE = {}
DEBUG = os.environ.get("KDBG", "0") == "1"


def _build_nc():
    nc = bacc.Bacc(
        "TRN2", target_bir_lowering=False, debug=False, num_devices=NCORES)
    xT_d = nc.dram_tensor("xT", [D, S], BF16, kind="ExternalInput")
    wq_d = nc.dram_tensor("wq", [D, D], BF16, kind="ExternalInput")
    wk_d = nc.dram_tensor("wk", [D, D], BF16, kind="ExternalInput")
    wv_d = nc.dram_tensor("wv", [D, D], BF16, kind="ExternalInput")
    wp_d = nc.dram_tensor("wp", [D, D], BF16, kind="ExternalInput")
    c1_d = nc.dram_tensor("c1", [P, S], F32, kind="ExternalInput")
    c2_d = nc.dram_tensor("c2", [P, S], F32, kind="ExternalInput")
    mask_d = nc.dram_tensor("mask", [P, P], BF16, kind="ExternalInput")
    ones_d = nc.dram_tensor("ones", [P, H], BF16, kind="ExternalInput")
    out_d = nc.dram_tensor("out", [S, D], F32, kind="ExternalOutput")
    if DEBUG:
        dbgq_d = nc.dram_tensor("dbgq", [D, S], BF16, kind="ExternalOutput")
        dbgk_d = nc.dram_tensor("dbgk", [D, S], BF16, kind="ExternalOutput")
        dbgy_d = nc.dram_tensor("dbgy", [D, S], BF16, kind="ExternalOutput")
        dbgv_d = nc.dram_tensor("dbgv", [S, H * (HS + 1)], BF16,
                                kind="ExternalOutput")

    def mm(out, lhsT, rhs, start, stop):
        nc.tensor.matmul(out, lhsT, rhs, start=start, stop=stop)

    with tile.TileContext(nc) as tc, ExitStack() as ctx:
        persist = ctx.enter_context(tc.tile_pool(name="persist", bufs=1))
        xt = [persist.tile([P, S], BF16, name=f"xt{i}", tag=f"xt{i}") for i in range(8)]
        qt = [persist.tile([P, S], BF16, name=f"qt{i}", tag=f"qt{i}") for i in range(8)]
        kt = [persist.tile([P, S], BF16, name=f"kt{i}", tag=f"kt{i}") for i in range(8)]
        yt = [persist.tile([P, S], BF16, name=f"yt{i}", tag=f"yt{i}") for i in range(8)]
        vt = [persist.tile([P, H, HS + 1], BF16, name=f"vt{i}", tag=f"vt{i}")
              for i in range(8)]
        wqs = [persist.tile([P, D], BF16, name=f"wqs{i}", tag=f"wqs{i}") for i in range(8)]
        wks = [persist.tile([P, D], BF16, name=f"wks{i}", tag=f"wks{i}") for i in range(8)]
        wvs = [persist.tile([P, D], BF16, name=f"wvs{i}", tag=f"wvs{i}") for i in range(8)]
        wps = [persist.tile([P, D], BF16, name=f"wps{i}", tag=f"wps{i}") for i in range(8)]
        c1 = persist.tile([P, S], F32, name="c1_t", tag="c1_t")
        c2 = persist.tile([P, S], F32, name="c2_t", tag="c2_t")
        maskt = persist.tile([P, P], BF16, name="maskt", tag="maskt")
        ones_t = persist.tile([P, H], BF16, name="ones_t", tag="ones_t")
        scratch = persist.tile([P, P], BF16, name="scratch", tag="scratch")

        # Warmup source must not depend on any DMA.
        nc.vector.memset(scratch[:], 0.0)

        # DMAs in order of first use: x & wv feed V, coeffs feed the first
        # rope, q/k weights feed the projection loop, wp only at the end.
        for i in range(8):
            nc.sync.dma_start(xt[i][:], xT_d[i * P:(i + 1) * P, :])
        for i in range(8):
            nc.sync.dma_start(wvs[i][:], wv_d[i * P:(i + 1) * P, :])
        for t, d_ in ((c1, c1_d), (c2, c2_d), (maskt, mask_d), (ones_t, ones_d)):
            nc.sync.dma_start(t[:], d_[:])
        for i in range(8):
            nc.sync.dma_start(wqs[i][:], wq_d[i * P:(i + 1) * P, :])
        for i in range(8):
            nc.sync.dma_start(wks[i][:], wk_d[i * P:(i + 1) * P, :])
        for i in range(8):
            nc.sync.dma_start(wps[i][:], wp_d[i * P:(i + 1) * P, :])

        with ExitStack() as mctx:
            pacc = mctx.enter_context(tc.tile_pool(name="pacc", bufs=2, space="PSUM"))
            pss_p = mctx.enter_context(tc.tile_pool(name="pss", bufs=4, space="PSUM"))
            psy_p = mctx.enter_context(tc.tile_pool(name="psy", bufs=2, space="PSUM"))
            attp = mctx.enter_context(tc.tile_pool(name="attp", bufs=18))
            rtmp = mctx.enter_context(tc.tile_pool(name="rtmp", bufs=4))
            smallp = mctx.enter_context(tc.tile_pool(name="smallp", bufs=6))

            # ---- PE warmup: lift the HAM clock gate while DMAs stream ----
            pw = pacc.tile([P, 512], F32, name="pw", tag="acc")
            for _ in range(64):
                nc.tensor.matmul(pw[:, 0:P], scratch[:], scratch[:], start=True,
                                 stop=True)

            # ---------------- V = x @ Wv ----------------
            for f2 in range(2):
                for sc in range(8):
                    ps = pacc.tile([P, 512], F32, name="vps", tag="acc")
                    for dc in range(8):
                        mm(ps[:], xt[dc][:, sc * P:(sc + 1) * P],
                           wvs[dc][:, f2 * 512:(f2 + 1) * 512], dc == 0, dc == 7)
                    nc.vector.tensor_copy(
                        vt[sc][:, f2 * 8:(f2 + 1) * 8, 0:HS],
                        ps[:].rearrange("p (h e) -> p h e", e=HS))
            for sc in range(8):
                nc.vector.tensor_copy(vt[sc][:, :, HS], ones_t[:])

            # ---------------- Q/K projection + RoPE ----------------
            def rope(ps, dst_slice, cc1, cc2, s0):
                # dst = ps * c1 + swap32(ps) * c2
                t = rtmp.tile([P, 512], F32, name="ropet", tag="rt")
                u = rtmp.tile([P, 512], F32, name="ropeu", tag="rt")
                for g, src in ((0, 32), (1, 0), (2, 96), (3, 64)):
                    nc.vector.tensor_copy(t[g * 32:(g + 1) * 32, :],
                                          ps[src:src + 32, :])
                nc.vector.tensor_mul(u[:], ps[:], cc1[:, s0:s0 + 512])
                nc.vector.tensor_mul(t[:], t[:], cc2[:, s0:s0 + 512])
                nc.vector.tensor_add(dst_slice, u[:], t[:])

            def qk_half(fc, which):
                wst, dst = (wqs, qt) if which == "q" else (wks, kt)
                for s2 in range(2):
                    ps = pacc.tile([P, 512], F32, name="qkps", tag="acc")
                    for dc in range(8):
                        mm(ps[:], wst[dc][:, fc * P:(fc + 1) * P],
                           xt[dc][:, s2 * 512:(s2 + 1) * 512], dc == 0, dc == 7)
                    rope(ps, dst[fc][:, s2 * 512:(s2 + 1) * 512], c1, c2,
                         s2 * 512)

            # ---------------- attention for head pair ft ----------------
            def scores_part(ft, qc):
                kmax = 4 if qc == 0 else 8
                atts = []
                for kc in range(kmax):
                    for hb in (0, 64):
                        pss = pss_p.tile([P, 512], F32, name="pss", tag="pss")
                        mm(pss[:], kt[ft][hb:hb + 64, kc * P:(kc + 1) * P],
                           qt[ft][hb:hb + 64, qc * 512:(qc + 1) * 512],
                           True, True)
                        att = attp.tile([P, 512], BF16, name="att", tag="att")
                        qsub = kc * P - qc * 512
                        if 0 <= qsub < 512:
                            if qsub > 0:
                                nc.vector.memset(att[:, 0:qsub], 0.0)
                            nc.scalar.activation(att[:, qsub:], pss[:, qsub:],
                                                 EXP, scale=SCALE)
                            nc.vector.tensor_mul(att[:, qsub:qsub + P],
                                                 att[:, qsub:qsub + P], maskt[:])
                        else:
                            nc.scalar.activation(att[:], pss[:], EXP, scale=SCALE)
                        atts.append(att)
                return atts

            def attv_part(ft, qc, atts):
                kmax = 4 if qc == 0 else 8
                psyA = psy_p.tile([HS + 1, 512], F32, name="psyA", tag="psy")
                psyB = psy_p.tile([HS + 1, 512], F32, name="psyB", tag="psy")
                for kc in range(kmax):
                    mm(psyA[:], vt[kc][:, 2 * ft, :], atts[2 * kc][:],
                       kc == 0, kc == kmax - 1)
                    mm(psyB[:], vt[kc][:, 2 * ft + 1, :], atts[2 * kc + 1][:],
                       kc == 0, kc == kmax - 1)
                for hb, psy in ((0, psyA), (64, psyB)):
                    sl = yt[ft][hb:hb + 64, qc * 512:(qc + 1) * 512]
                    srow = smallp.tile([1, 512], F32, name="srow", tag="rr")
                    nc.vector.tensor_copy(srow[:], psy[HS:HS + 1, :])
                    nc.vector.tensor_copy(sl, psy[0:HS, :])
                    rb = smallp.tile([P, 512], F32, name="rb", tag="rb")
                    nc.gpsimd.partition_broadcast(rb[:], srow[0:1, :])
                    nc.vector.reciprocal_approx_fast(out=rb[:], in_=rb[:])
                    nc.vector.tensor_mul(sl, sl, rb[hb:hb + 64, :])

            qk_half(0, "q")
            qk_half(0, "k")
            for fc in range(8):
                a0 = scores_part(fc, 0)
                if fc < 7:
                    qk_half(fc + 1, "q")
                attv_part(fc, 0, a0)
                a1 = scores_part(fc, 1)
                if fc < 7:
                    qk_half(fc + 1, "k")
                attv_part(fc, 1, a1)

        if DEBUG:
            for i in range(8):
                nc.sync.dma_start(dbgq_d[i * P:(i + 1) * P, :], qt[i][:])
                nc.sync.dma_start(dbgk_d[i * P:(i + 1) * P, :], kt[i][:])
                nc.sync.dma_start(dbgy_d[i * P:(i + 1) * P, :], yt[i][:])
                nc.sync.dma_start(
                    dbgv_d[i * P:(i + 1) * P, :],
                    vt[i][:].rearrange("p h e -> p (h e)"))

        # ---------------- out = y @ Wp ----------------
        with ExitStack() as dctx:
            outp = dctx.enter_context(tc.tile_pool(name="outp", bufs=4))
            psp_p = dctx.enter_context(tc.tile_pool(name="psp", bufs=3, space="PSUM"))
            for n2 in range(2):
                for sc in range(8):
                    psp = psp_p.tile([P, 512], F32, name="psp", tag="psp")
                    for dc in range(8):
                        mm(psp[:], yt[dc][:, sc * P:(sc + 1) * P],
                           wps[dc][:, n2 * 512:(n2 + 1) * 512], dc == 0, dc == 7)
                    ot = outp.tile([P, 512], F32, name="ot", tag="ot")
                    nc.scalar.copy(ot[:], psp[:])
                    nc.sync.dma_start(
                        out_d[sc * P:(sc + 1) * P, n2 * 512:(n2 + 1) * 512], ot[:])
    nc.compile()
    return nc


def _prep(inputs):
    bf = ml_dtypes.bfloat16
    w_qkv = np.asarray(inputs["w_qkv"], np.float32)
    w_proj = np.asarray(inputs["w_proj"], np.float32)
    cos = np.asarray(inputs["cos"], np.float32).reshape(S, HS // 2)
    sin = np.asarray(inputs["sin"], np.float32).reshape(S, HS // 2)
    wq, wk, wv = w_qkv[:, 0:D], w_qkv[:, D:2 * D], w_qkv[:, 2 * D:3 * D]
    perm = np.empty(D, np.int64)
    for h in range(H):
        b0 = h * HS
        perm[b0:b0 + HS // 2] = b0 + np.arange(0, HS, 2)
        perm[b0 + HS // 2:b0 + HS] = b0 + np.arange(1, HS, 2)
    wq, wk = wq[:, perm], wk[:, perm]
    cosT = np.ascontiguousarray(cos.T)  # [32, S]
    sinT = np.ascontiguousarray(sin.T)
    c1 = np.concatenate([cosT, cosT, cosT, cosT], 0)  # [128, S]
    c2 = np.concatenate([-sinT, sinT, -sinT, sinT], 0)
    mask = np.triu(np.ones((P, P), np.float32))  # [k, q]: allow q >= k
    common = {
        "wq": np.ascontiguousarray(wq).astype(bf),
        "wk": np.ascontiguousarray(wk).astype(bf),
        "wv": np.ascontiguousarray(wv).astype(bf),
        "wp": np.ascontiguousarray(w_proj).astype(bf),
        "c1": c1, "c2": c2, "mask": mask.astype(bf),
        "ones": np.ones((P, H), np.float32).astype(bf),
    }
    return common


LAST_RESULT = None


def kernel(**inputs):
    global LAST_RESULT
    if "nc" not in _CACHE:
        _CACHE["nc"] = _build_nc()
    nc = _CACHE["nc"]
    common = _prep(inputs)
    bf = ml_dtypes.bfloat16
    x = np.asarray(inputs["x"], np.float32)
    in_maps = [
        dict(common, xT=np.ascontiguousarray(x[b].T).astype(bf))
        for b in range(B)
    ]
    res = run_bass_kernel_spmd(nc, in_maps, list(range(NCORES)))
    LAST_RESULT = res
    out = np.stack([res.results[i]["out"] for i in range(B)], 0)
    return out.astype(np.float32)


# revision 19
# speedup vs baseline: 1.6814x; 1.4216x over previous
"""Causal self-attention (RoPE, 16 heads) Trainium2 Bass kernel.

Problem: B=8, S=1024, D=1024, H=16, HS=64, fp32 in/out, causal + all-ones mask.

Strategy: data-parallel over batch — one batch element per NeuronCore (8 cores).
All matmuls in bf16 (fp32 PSUM accumulation); fp32 only for PSUM, softmax
reciprocal, and the final output.

Per-core layout ("transposed activations", no on-chip transposes at all —
x^T is produced on the host):

  x^T   [D, S] bf16  host-transposed, DMA'd straight into SBUF
  Q^T,K^T [D, S]     = W^T @ x^T (lhsT = W tiles), RoPE applied via
                       deinterleaved-head column permutation of W (host) +
                       cos/sin coefficient tiles; swap-halves via DVE
                       partition-offset copies (off the ACT engine)
  V     [S, D]       = x @ W_v, stored per-head with an appended ones column
                       so att@v also yields the softmax denominators
  S^T   [k, q]       = K^T-chunks @ Q^T per head, causal blocks only; the
                       1/sqrt(hs) scale is folded into exp's scale immediate
  att   bf16         = exp(S^T) (no max subtraction: |scores| is small);
                       diagonal blocks masked by a 0/1 triangle multiply
  y^T   [D, S]       accumulated per head; row 64 = denominators; normalize
                       with reciprocal + gpsimd partition broadcast, fused
                       into the PSUM->SBUF move
  out   [S, D] fp32  = y @ W_proj

Emission order interleaves the attention of head-pair fc with the QKV
projection of fc+1 so the PE never idles long enough for the HAM clock gate
to re-throttle it; a burst of warmup matmuls at t=0 lifts the gate while the
initial DMAs stream in.
"""

import os

# The Bass kernel executes through the axon PJRT backend and needs the
# NeuronCores visible; a JAX_PLATFORMS=cpu pin (used for jax reference
# computation) would hide them.
if "axon" not in os.environ.get("JAX_PLATFORMS", "axon"):
    os.environ.pop("JAX_PLATFORMS", None)

import numpy as np
import ml_dtypes
from contextlib import ExitStack

import concourse.bass as bass
import concourse.mybir as mybir
import concourse.tile as tile
from concourse import bacc
from concourse.bass_utils import run_bass_kernel_spmd

B, S, D, H, HS = 8, 1024, 1024, 16, 64
P = 128
NCORES = 8
F32 = mybir.dt.float32
BF16 = mybir.dt.bfloat16
EXP = mybir.ActivationFunctionType.Exp
SCALE = 0.125  # 1/sqrt(HS)

_CACHE = {}
DEBUG = os.environ.get("KDBG", "0") == "1"


def _build_nc():
    nc = bacc.Bacc(
        "TRN2", target_bir_lowering=False, debug=False, num_devices=NCORES)
    xT_d = nc.dram_tensor("xT", [D, S], BF16, kind="ExternalInput")
    wq_d = nc.dram_tensor("wq", [D, D], BF16, kind="ExternalInput")
    wk_d = nc.dram_tensor("wk", [D, D], BF16, kind="ExternalInput")
    wv_d = nc.dram_tensor("wv", [D, D], BF16, kind="ExternalInput")
    wp_d = nc.dram_tensor("wp", [D, D], BF16, kind="ExternalInput")
    c1_d = nc.dram_tensor("c1", [P, S], BF16, kind="ExternalInput")
    c2_d = nc.dram_tensor("c2", [P, S], BF16, kind="ExternalInput")
    mask_d = nc.dram_tensor("mask", [P, P], BF16, kind="ExternalInput")
    ones_d = nc.dram_tensor("ones", [P, H], BF16, kind="ExternalInput")
    out_d = nc.dram_tensor("out", [S, D], F32, kind="ExternalOutput")
    if DEBUG:
        dbgq_d = nc.dram_tensor("dbgq", [D, S], BF16, kind="ExternalOutput")
        dbgk_d = nc.dram_tensor("dbgk", [D, S], BF16, kind="ExternalOutput")
        dbgy_d = nc.dram_tensor("dbgy", [D, S], BF16, kind="ExternalOutput")
        dbgv_d = nc.dram_tensor("dbgv", [S, H * (HS + 1)], BF16,
                                kind="ExternalOutput")

    def mm(out, lhsT, rhs, start, stop):
        nc.tensor.matmul(out, lhsT, rhs, start=start, stop=stop)

    with tile.TileContext(nc) as tc, ExitStack() as ctx:
        persist = ctx.enter_context(tc.tile_pool(name="persist", bufs=1))
        xt = [persist.tile([P, S], BF16, name=f"xt{i}", tag=f"xt{i}") for i in range(8)]
        qt = [persist.tile([P, S], BF16, name=f"qt{i}", tag=f"qt{i}") for i in range(8)]
        kt = [persist.tile([P, S], BF16, name=f"kt{i}", tag=f"kt{i}") for i in range(8)]
        yt = [persist.tile([P, S], BF16, name=f"yt{i}", tag=f"yt{i}") for i in range(8)]
        vt = [persist.tile([P, H, HS + 1], BF16, name=f"vt{i}", tag=f"vt{i}")
              for i in range(8)]
        wqs = [persist.tile([P, D], BF16, name=f"wqs{i}", tag=f"wqs{i}") for i in range(8)]
        wks = [persist.tile([P, D], BF16, name=f"wks{i}", tag=f"wks{i}") for i in range(8)]
        wvs = [persist.tile([P, D], BF16, name=f"wvs{i}", tag=f"wvs{i}") for i in range(8)]
        wps = [persist.tile([P, D], BF16, name=f"wps{i}", tag=f"wps{i}") for i in range(8)]
        c1 = persist.tile([P, S], BF16, name="c1_t", tag="c1_t")
        c2 = persist.tile([P, S], BF16, name="c2_t", tag="c2_t")
        maskt = persist.tile([P, P], BF16, name="maskt", tag="maskt")
        ones_t = persist.tile([P, H], BF16, name="ones_t", tag="ones_t")
        scratch = persist.tile([P, P], BF16, name="scratch", tag="scratch")

        # Warmup source must not depend on any DMA.
        nc.vector.memset(scratch[:], 0.0)

        # DMAs in order of first use: x & wv feed V, coeffs feed the first
        # rope, q/k weights feed the projection loop, wp only at the end.
        for i in range(8):
            nc.sync.dma_start(xt[i][:], xT_d[i * P:(i + 1) * P, :])
        for i in range(8):
            nc.sync.dma_start(wvs[i][:], wv_d[i * P:(i + 1) * P, :])
        for t, d_ in ((c1, c1_d), (c2, c2_d), (maskt, mask_d), (ones_t, ones_d)):
            nc.sync.dma_start(t[:], d_[:])
        for i in range(8):
            nc.sync.dma_start(wqs[i][:], wq_d[i * P:(i + 1) * P, :])
        for i in range(8):
            nc.sync.dma_start(wks[i][:], wk_d[i * P:(i + 1) * P, :])
        for i in range(8):
            nc.sync.dma_start(wps[i][:], wp_d[i * P:(i + 1) * P, :])

        with ExitStack() as mctx:
            pacc = mctx.enter_context(tc.tile_pool(name="pacc", bufs=2, space="PSUM"))
            pss_p = mctx.enter_context(tc.tile_pool(name="pss", bufs=4, space="PSUM"))
            psy_p = mctx.enter_context(tc.tile_pool(name="psy", bufs=2, space="PSUM"))
            attp = mctx.enter_context(tc.tile_pool(name="attp", bufs=18))
            rtmp = mctx.enter_context(tc.tile_pool(name="rtmp", bufs=4))
            smallp = mctx.enter_context(tc.tile_pool(name="smallp", bufs=6))

            # ---- PE warmup: lift the HAM clock gate while DMAs stream ----
            pw = pacc.tile([P, 512], F32, name="pw", tag="acc")
            for _ in range(64):
                nc.tensor.matmul(pw[:, 0:P], scratch[:], scratch[:], start=True,
                                 stop=True)

            # ---------------- V = x @ Wv ----------------
            for f2 in range(2):
                for sc in range(8):
                    ps = pacc.tile([P, 512], F32, name="vps", tag="acc")
                    for dc in range(8):
                        mm(ps[:], xt[dc][:, sc * P:(sc + 1) * P],
                           wvs[dc][:, f2 * 512:(f2 + 1) * 512], dc == 0, dc == 7)
                    nc.vector.tensor_copy(
                        vt[sc][:, f2 * 8:(f2 + 1) * 8, 0:HS],
                        ps[:].rearrange("p (h e) -> p h e", e=HS))
            for sc in range(8):
                nc.vector.tensor_copy(vt[sc][:, :, HS], ones_t[:])

            # ---------------- Q/K projection + RoPE ----------------
            def rope(ps, dst_slice, cc1, cc2, s0):
                # dst = ps * c1 + swap32(ps) * c2.  The swap-halves shuffle is
                # done by the (otherwise idle) DMA engines SBUF->SBUF; the
                # PSUM bank is released after the single ACT copy.
                raw = rtmp.tile([P, 512], BF16, name="ropraw", tag="raw")
                nc.scalar.copy(raw[:], ps[:])
                t = rtmp.tile([P, 512], BF16, name="ropet", tag="rt")
                for g, src in ((0, 32), (1, 0), (2, 96), (3, 64)):
                    nc.sync.dma_start(t[g * 32:(g + 1) * 32, :],
                                      raw[src:src + 32, :])
                u = rtmp.tile([P, 512], BF16, name="ropeu", tag="ru")
                nc.vector.tensor_mul(u[:], raw[:], cc1[:, s0:s0 + 512])
                nc.vector.tensor_mul(t[:], t[:], cc2[:, s0:s0 + 512])
                nc.vector.tensor_add(dst_slice, u[:], t[:])

            def qk_half(fc, which):
                wst, dst = (wqs, qt) if which == "q" else (wks, kt)
                for s2 in range(2):
                    ps = pacc.tile([P, 512], F32, name="qkps", tag="acc")
                    for dc in range(8):
                        mm(ps[:], wst[dc][:, fc * P:(fc + 1) * P],
                           xt[dc][:, s2 * 512:(s2 + 1) * 512], dc == 0, dc == 7)
                    rope(ps, dst[fc][:, s2 * 512:(s2 + 1) * 512], c1, c2,
                         s2 * 512)

            # ---------------- attention for head pair ft ----------------
            def scores_part(ft, qc):
                kmax = 4 if qc == 0 else 8
                atts = []
                for kc in range(kmax):
                    for hb in (0, 64):
                        pss = pss_p.tile([P, 512], F32, name="pss", tag="pss")
                        mm(pss[:], kt[ft][hb:hb + 64, kc * P:(kc + 1) * P],
                           qt[ft][hb:hb + 64, qc * 512:(qc + 1) * 512],
                           True, True)
                        att = attp.tile([P, 512], BF16, name="att", tag="att")
                        qsub = kc * P - qc * 512
                        if 0 <= qsub < 512:
                            nc.scalar.activation(att[:, qsub:], pss[:, qsub:],
                                                 EXP, scale=SCALE)
                            nc.vector.tensor_mul(att[:, qsub:qsub + P],
                                                 att[:, qsub:qsub + P], maskt[:])
                            atts.append((att, qsub))
                        else:
                            nc.scalar.activation(att[:], pss[:], EXP, scale=SCALE)
                            atts.append((att, 0))
                return atts

            def attv_part(ft, qc, atts):
                # ascending kc: the causally-valid span only narrows, so every
                # accumulation lands inside the start=True footprint and the
                # never-written columns are never read.
                kmax = 4 if qc == 0 else 8
                psyA = psy_p.tile([HS + 1, 512], F32, name="psyA", tag="psy")
                psyB = psy_p.tile([HS + 1, 512], F32, name="psyB", tag="psy")
                for kc in range(kmax):
                    attA, loA = atts[2 * kc]
                    attB, loB = atts[2 * kc + 1]
                    nc.tensor.matmul(
                        psyA[:, loA:], vt[kc][:, 2 * ft, :], attA[:, loA:],
                        start=kc == 0, stop=kc == kmax - 1,
                        skip_group_check=True)
                    nc.tensor.matmul(
                        psyB[:, loB:], vt[kc][:, 2 * ft + 1, :], attB[:, loB:],
                        start=kc == 0, stop=kc == kmax - 1,
                        skip_group_check=True)
                for hb, psy in ((0, psyA), (64, psyB)):
                    sl = yt[ft][hb:hb + 64, qc * 512:(qc + 1) * 512]
                    srow = smallp.tile([1, 512], F32, name="srow", tag="rr")
                    nc.vector.tensor_copy(srow[:], psy[HS:HS + 1, :])
                    rb = smallp.tile([P, 512], F32, name="rb", tag="rb")
                    nc.gpsimd.partition_broadcast(rb[:], srow[0:1, :])
                    nc.vector.reciprocal_approx_fast(out=rb[:], in_=rb[:])
                    nc.vector.tensor_mul(sl, psy[0:HS, :], rb[hb:hb + 64, :])

            qk_half(0, "q")
            qk_half(0, "k")
            for fc in range(8):
                a0 = scores_part(fc, 0)
                if fc < 7:
                    qk_half(fc + 1, "q")
                attv_part(fc, 0, a0)
                a1 = scores_part(fc, 1)
                if fc < 7:
                    qk_half(fc + 1, "k")
                attv_part(fc, 1, a1)

        if DEBUG:
            for i in range(8):
                nc.sync.dma_start(dbgq_d[i * P:(i + 1) * P, :], qt[i][:])
                nc.sync.dma_start(dbgk_d[i * P:(i + 1) * P, :], kt[i][:])
                nc.sync.dma_start(dbgy_d[i * P:(i + 1) * P, :], yt[i][:])
                nc.sync.dma_start(
                    dbgv_d[i * P:(i + 1) * P, :],
                    vt[i][:].rearrange("p h e -> p (h e)"))

        # ---------------- out = y @ Wp ----------------
        with ExitStack() as dctx:
            outp = dctx.enter_context(tc.tile_pool(name="outp", bufs=4))
            psp_p = dctx.enter_context(tc.tile_pool(name="psp", bufs=3, space="PSUM"))
            for n2 in range(2):
                for sc in range(8):
                    psp = psp_p.tile([P, 512], F32, name="psp", tag="psp")
                    for dc in range(8):
                        mm(psp[:], yt[dc][:, sc * P:(sc + 1) * P],
                           wps[dc][:, n2 * 512:(n2 + 1) * 512], dc == 0, dc == 7)
                    ot = outp.tile([P, 512], F32, name="ot", tag="ot")
                    nc.scalar.copy(ot[:], psp[:])
                    nc.sync.dma_start(
                        out_d[sc * P:(sc + 1) * P, n2 * 512:(n2 + 1) * 512], ot[:])
    nc.compile()
    return nc


def _prep(inputs):
    bf = ml_dtypes.bfloat16
    w_qkv = np.asarray(inputs["w_qkv"], np.float32)
    w_proj = np.asarray(inputs["w_proj"], np.float32)
    cos = np.asarray(inputs["cos"], np.float32).reshape(S, HS // 2)
    sin = np.asarray(inputs["sin"], np.float32).reshape(S, HS // 2)
    wq, wk, wv = w_qkv[:, 0:D], w_qkv[:, D:2 * D], w_qkv[:, 2 * D:3 * D]
    perm = np.empty(D, np.int64)
    for h in range(H):
        b0 = h * HS
        perm[b0:b0 + HS // 2] = b0 + np.arange(0, HS, 2)
        perm[b0 + HS // 2:b0 + HS] = b0 + np.arange(1, HS, 2)
    wq, wk = wq[:, perm], wk[:, perm]
    cosT = np.ascontiguousarray(cos.T)  # [32, S]
    sinT = np.ascontiguousarray(sin.T)
    c1 = np.concatenate([cosT, cosT, cosT, cosT], 0)  # [128, S]
    c2 = np.concatenate([-sinT, sinT, -sinT, sinT], 0)
    mask = np.triu(np.ones((P, P), np.float32))  # [k, q]: allow q >= k
    common = {
        "wq": np.ascontiguousarray(wq).astype(bf),
        "wk": np.ascontiguousarray(wk).astype(bf),
        "wv": np.ascontiguousarray(wv).astype(bf),
        "wp": np.ascontiguousarray(w_proj).astype(bf),
        "c1": c1.astype(bf), "c2": c2.astype(bf), "mask": mask.astype(bf),
        "ones": np.ones((P, H), np.float32).astype(bf),
    }
    return common


LAST_RESULT = None


def kernel(**inputs):
    global LAST_RESULT
    if "nc" not in _CACHE:
        _CACHE["nc"] = _build_nc()
    nc = _CACHE["nc"]
    common = _prep(inputs)
    bf = ml_dtypes.bfloat16
    x = np.asarray(inputs["x"], np.float32)
    in_maps = [
        dict(common, xT=np.ascontiguousarray(x[b].T).astype(bf))
        for b in range(B)
    ]
    res = run_bass_kernel_spmd(nc, in_maps, list(range(NCORES)))
    LAST_RESULT = res
    out = np.stack([res.results[i]["out"] for i in range(B)], 0)
    return out.astype(np.float32)


# revision 22
# speedup vs baseline: 1.8463x; 1.0981x over previous
"""Causal self-attention (RoPE, 16 heads) Trainium2 Bass kernel.

Problem: B=8, S=1024, D=1024, H=16, HS=64, fp32 in/out, causal + all-ones mask.

Strategy: data-parallel over batch — one batch element per NeuronCore (8 cores).
All matmuls in bf16 (fp32 PSUM accumulation); fp32 only for PSUM, softmax
reciprocal, and the final output.

Per-core layout ("transposed activations", no on-chip transposes at all —
x^T is produced on the host):

  x^T   [D, S] bf16  host-transposed, DMA'd straight into SBUF
  Q^T,K^T [D, S]     = W^T @ x^T (lhsT = W tiles), RoPE applied via
                       deinterleaved-head column permutation of W (host) +
                       cos/sin coefficient tiles; swap-halves via DVE
                       partition-offset copies (off the ACT engine)
  V     [S, D]       = x @ W_v, stored per-head with an appended ones column
                       so att@v also yields the softmax denominators
  S^T   [k, q]       = K^T-chunks @ Q^T per head, causal blocks only; the
                       1/sqrt(hs) scale is folded into exp's scale immediate
  att   bf16         = exp(S^T) (no max subtraction: |scores| is small);
                       diagonal blocks masked by a 0/1 triangle multiply
  y^T   [D, S]       accumulated per head; row 64 = denominators; normalize
                       with reciprocal + gpsimd partition broadcast, fused
                       into the PSUM->SBUF move
  out   [S, D] fp32  = y @ W_proj

Emission order interleaves the attention of head-pair fc with the QKV
projection of fc+1 so the PE never idles long enough for the HAM clock gate
to re-throttle it; a burst of warmup matmuls at t=0 lifts the gate while the
initial DMAs stream in.
"""

import os

# The Bass kernel executes through the axon PJRT backend and needs the
# NeuronCores visible; a JAX_PLATFORMS=cpu pin (used for jax reference
# computation) would hide them.
if "axon" not in os.environ.get("JAX_PLATFORMS", "axon"):
    os.environ.pop("JAX_PLATFORMS", None)

import numpy as np
import ml_dtypes
from contextlib import ExitStack

import concourse.bass as bass
import concourse.mybir as mybir
import concourse.tile as tile
from concourse import bacc
from concourse.bass_utils import run_bass_kernel_spmd

B, S, D, H, HS = 8, 1024, 1024, 16, 64
P = 128
NCORES = 8
F32 = mybir.dt.float32
BF16 = mybir.dt.bfloat16
EXP = mybir.ActivationFunctionType.Exp
SCALE = 0.125  # 1/sqrt(HS)

_CACHE = {}
DEBUG = os.environ.get("KDBG", "0") == "1"


def _build_nc():
    nc = bacc.Bacc(
        "TRN2", target_bir_lowering=False, debug=False, num_devices=NCORES)
    xT_d = nc.dram_tensor("xT", [D, S], BF16, kind="ExternalInput")
    wq_d = nc.dram_tensor("wq", [D, D], BF16, kind="ExternalInput")
    wk_d = nc.dram_tensor("wk", [D, D], BF16, kind="ExternalInput")
    wv_d = nc.dram_tensor("wv", [D, D], BF16, kind="ExternalInput")
    wp_d = nc.dram_tensor("wp", [D, D], BF16, kind="ExternalInput")
    c1_d = nc.dram_tensor("c1", [P, S], BF16, kind="ExternalInput")
    c2_d = nc.dram_tensor("c2", [P, S], BF16, kind="ExternalInput")
    mask_d = nc.dram_tensor("mask", [P, P], BF16, kind="ExternalInput")
    ones_d = nc.dram_tensor("ones", [P, H], BF16, kind="ExternalInput")
    out_d = nc.dram_tensor("out", [S, D], F32, kind="ExternalOutput")
    if DEBUG:
        dbgq_d = nc.dram_tensor("dbgq", [D, S], BF16, kind="ExternalOutput")
        dbgk_d = nc.dram_tensor("dbgk", [D, S], BF16, kind="ExternalOutput")
        dbgy_d = nc.dram_tensor("dbgy", [D, S], BF16, kind="ExternalOutput")
        dbgv_d = nc.dram_tensor("dbgv", [S, H * (HS + 1)], BF16,
                                kind="ExternalOutput")

    def mm(out, lhsT, rhs, start, stop):
        nc.tensor.matmul(out, lhsT, rhs, start=start, stop=stop)

    with tile.TileContext(nc) as tc, ExitStack() as ctx:
        persist = ctx.enter_context(tc.tile_pool(name="persist", bufs=1))
        xt = [persist.tile([P, S], BF16, name=f"xt{i}", tag=f"xt{i}") for i in range(8)]
        qt = [persist.tile([P, S], BF16, name=f"qt{i}", tag=f"qt{i}") for i in range(8)]
        kt = [persist.tile([P, S], BF16, name=f"kt{i}", tag=f"kt{i}") for i in range(8)]
        yt = [persist.tile([P, S], BF16, name=f"yt{i}", tag=f"yt{i}") for i in range(8)]
        vt = [persist.tile([P, H, HS + 1], BF16, name=f"vt{i}", tag=f"vt{i}")
              for i in range(8)]
        wqs = [persist.tile([P, D], BF16, name=f"wqs{i}", tag=f"wqs{i}") for i in range(8)]
        wks = [persist.tile([P, D], BF16, name=f"wks{i}", tag=f"wks{i}") for i in range(8)]
        wvs = [persist.tile([P, D], BF16, name=f"wvs{i}", tag=f"wvs{i}") for i in range(8)]
        wps = [persist.tile([P, D], BF16, name=f"wps{i}", tag=f"wps{i}") for i in range(8)]
        c1 = persist.tile([P, S], BF16, name="c1_t", tag="c1_t")
        c2 = persist.tile([P, S], BF16, name="c2_t", tag="c2_t")
        maskt = persist.tile([P, P], BF16, name="maskt", tag="maskt")
        ones_t = persist.tile([P, H], BF16, name="ones_t", tag="ones_t")
        scratch = persist.tile([P, P], BF16, name="scratch", tag="scratch")

        # Warmup source must not depend on any DMA.
        nc.vector.memset(scratch[:], 0.0)

        # DMAs in order of first use: x & wv feed V, coeffs feed the first
        # rope, q/k weights feed the projection loop, wp only at the end.
        for i in range(8):
            nc.sync.dma_start(xt[i][:], xT_d[i * P:(i + 1) * P, :])
        for i in range(8):
            nc.sync.dma_start(wvs[i][:], wv_d[i * P:(i + 1) * P, :])
        for t, d_ in ((c1, c1_d), (c2, c2_d), (maskt, mask_d), (ones_t, ones_d)):
            nc.sync.dma_start(t[:], d_[:])
        for i in range(8):
            nc.sync.dma_start(wqs[i][:], wq_d[i * P:(i + 1) * P, :])
        for i in range(8):
            nc.sync.dma_start(wks[i][:], wk_d[i * P:(i + 1) * P, :])
        for i in range(8):
            nc.sync.dma_start(wps[i][:], wp_d[i * P:(i + 1) * P, :])

        with ExitStack() as mctx:
            pacc = mctx.enter_context(tc.tile_pool(name="pacc", bufs=2, space="PSUM"))
            pss_p = mctx.enter_context(tc.tile_pool(name="pss", bufs=2, space="PSUM"))
            psy_p = mctx.enter_context(tc.tile_pool(name="psy", bufs=1, space="PSUM"))
            attp = mctx.enter_context(tc.tile_pool(name="attp", bufs=9))
            rtmp = mctx.enter_context(tc.tile_pool(name="rtmp", bufs=3))
            smallp = mctx.enter_context(tc.tile_pool(name="smallp", bufs=2))

            # ---- PE warmup: lift the HAM clock gate while DMAs stream ----
            pw = pacc.tile([P, 512], F32, name="pw", tag="acc")
            for _ in range(64):
                nc.tensor.matmul(pw[:, 0:P], scratch[:], scratch[:], start=True,
                                 stop=True)

            # ---------------- V = x @ Wv ----------------
            for f2 in range(2):
                for sc in range(8):
                    ps = pacc.tile([P, 512], F32, name="vps", tag="acc")
                    for dc in range(8):
                        mm(ps[:], xt[dc][:, sc * P:(sc + 1) * P],
                           wvs[dc][:, f2 * 512:(f2 + 1) * 512], dc == 0, dc == 7)
                    nc.vector.tensor_copy(
                        vt[sc][:, f2 * 8:(f2 + 1) * 8, 0:HS],
                        ps[:].rearrange("p (h e) -> p h e", e=HS))
            for sc in range(8):
                nc.vector.tensor_copy(vt[sc][:, :, HS], ones_t[:])

            # ---------------- Q/K projection + RoPE ----------------
            def rope(ps, dst_slice, cc1, cc2, s0):
                # dst = ps * c1 + swap32(ps) * c2.  The swap-halves shuffle is
                # done by the (otherwise idle) DMA engines SBUF->SBUF; the
                # PSUM bank is released after the single ACT copy.
                raw = rtmp.tile([P, 512], BF16, name="ropraw", tag="raw")
                nc.scalar.copy(raw[:], ps[:])
                t = rtmp.tile([P, 512], BF16, name="ropet", tag="rt")
                for g, src in ((0, 32), (1, 0), (2, 96), (3, 64)):
                    nc.sync.dma_start(t[g * 32:(g + 1) * 32, :],
                                      raw[src:src + 32, :])
                u = rtmp.tile([P, 512], BF16, name="ropeu", tag="ru")
                nc.vector.tensor_mul(u[:], raw[:], cc1[:, s0:s0 + 512])
                nc.vector.tensor_mul(t[:], t[:], cc2[:, s0:s0 + 512])
                nc.vector.tensor_add(dst_slice, u[:], t[:])

            def qk_half(fc, which):
                wst, dst = (wqs, qt) if which == "q" else (wks, kt)
                for s2 in range(2):
                    ps = pacc.tile([P, 512], F32, name="qkps", tag="acc")
                    for dc in range(8):
                        mm(ps[:], wst[dc][:, fc * P:(fc + 1) * P],
                           xt[dc][:, s2 * 512:(s2 + 1) * 512], dc == 0, dc == 7)
                    rope(ps, dst[fc][:, s2 * 512:(s2 + 1) * 512], c1, c2,
                         s2 * 512)

            # ---------------- attention for head pair ft ----------------
            # Both heads (hb=0/64) of a kc block share one 2-bank PSUM tile:
            # one exp, one sums row, one reciprocal+broadcast per pair.
            def score_pair(ft, qc, kc):
                pss = pss_p.tile([P, 1024], F32, name="pss", tag="pss")
                for hb in (0, 64):
                    nc.tensor.matmul(
                        pss[:, hb * 8:hb * 8 + 512],
                        kt[ft][hb:hb + 64, kc * P:(kc + 1) * P],
                        qt[ft][hb:hb + 64, qc * 512:(qc + 1) * 512],
                        start=True, stop=True, skip_group_check=True)
                att = attp.tile([P, 1024], BF16, name="att", tag="att")
                qsub = kc * P - qc * 512
                if 0 <= qsub < 512:
                    nc.scalar.activation(att[:, qsub:], pss[:, qsub:],
                                         EXP, scale=SCALE)
                    nc.vector.tensor_mul(att[:, qsub:qsub + P],
                                         att[:, qsub:qsub + P], maskt[:])
                    nc.vector.tensor_mul(att[:, 512 + qsub:512 + qsub + P],
                                         att[:, 512 + qsub:512 + qsub + P],
                                         maskt[:])
                    return (att, qsub)
                nc.scalar.activation(att[:], pss[:], EXP, scale=SCALE)
                return (att, 0)

            def attv_part(ft, qc, atts):
                # ascending kc: the causally-valid span only narrows, so every
                # accumulation lands inside the start=True footprint and the
                # never-written columns are never read.
                kmax = 4 if qc == 0 else 8
                psy = psy_p.tile([HS + 1, 1024], F32, name="psy", tag="psy")
                for kc in range(kmax):
                    att, lo = atts[kc]
                    nc.tensor.matmul(
                        psy[:, lo:512], vt[kc][:, 2 * ft, :], att[:, lo:512],
                        start=kc == 0, stop=kc == kmax - 1,
                        skip_group_check=True)
                    nc.tensor.matmul(
                        psy[:, 512 + lo:], vt[kc][:, 2 * ft + 1, :],
                        att[:, 512 + lo:],
                        start=kc == 0, stop=kc == kmax - 1,
                        skip_group_check=True)
                srow = smallp.tile([1, 1024], F32, name="srow", tag="rr")
                nc.vector.tensor_copy(srow[:], psy[HS:HS + 1, :])
                nc.vector.reciprocal_approx_fast(out=srow[:], in_=srow[:])
                rb = smallp.tile([P, 1024], F32, name="rb", tag="rb")
                nc.gpsimd.partition_broadcast(rb[:], srow[0:1, :])
                nc.vector.tensor_mul(
                    yt[ft][0:64, qc * 512:(qc + 1) * 512],
                    psy[0:HS, 0:512], rb[0:64, 0:512])
                nc.vector.tensor_mul(
                    yt[ft][64:128, qc * 512:(qc + 1) * 512],
                    psy[0:HS, 512:1024], rb[64:128, 512:1024])

            def qk_mms(fc, which):
                # yields the 16 projection matmuls + the 2 rope tails as
                # closures, so the caller can interleave them with score work.
                wst, dst = (wqs, qt) if which == "q" else (wks, kt)
                units = []
                pss_tiles = {}

                def proj_mm(s2, dc):
                    def emit():
                        if dc == 0:
                            pss_tiles[s2] = pacc.tile(
                                [P, 512], F32, name="qkps", tag="acc")
                        nc.tensor.matmul(
                            pss_tiles[s2][:],
                            wst[dc][:, fc * P:(fc + 1) * P],
                            xt[dc][:, s2 * 512:(s2 + 1) * 512],
                            start=dc == 0, stop=dc == 7,
                            skip_group_check=True)
                        if dc == 7:
                            rope(pss_tiles[s2],
                                 dst[fc][:, s2 * 512:(s2 + 1) * 512],
                                 c1, c2, s2 * 512)
                    return emit

                for s2 in range(2):
                    for dc in range(8):
                        units.append(proj_mm(s2, dc))
                return units

            def run_units(units, n):
                for _ in range(n):
                    if units:
                        units.pop(0)()

            # prologue: Q(0), K(0)
            for u in qk_mms(0, "q"):
                u()
            for u in qk_mms(0, "k"):
                u()
            for fc in range(8):
                qu = qk_mms(fc + 1, "q") if fc < 7 else []
                ku = qk_mms(fc + 1, "k") if fc < 7 else []
                a0 = []
                for kc in range(4):
                    a0.append(score_pair(fc, 0, kc))
                    run_units(qu, 4)
                run_units(qu, 16)
                attv_part(fc, 0, a0)
                a1 = []
                for kc in range(8):
                    a1.append(score_pair(fc, 1, kc))
                    run_units(ku, 2)
                run_units(ku, 16)
                attv_part(fc, 1, a1)

        if DEBUG:
            for i in range(8):
                nc.sync.dma_start(dbgq_d[i * P:(i + 1) * P, :], qt[i][:])
                nc.sync.dma_start(dbgk_d[i * P:(i + 1) * P, :], kt[i][:])
                nc.sync.dma_start(dbgy_d[i * P:(i + 1) * P, :], yt[i][:])
                nc.sync.dma_start(
                    dbgv_d[i * P:(i + 1) * P, :],
                    vt[i][:].rearrange("p h e -> p (h e)"))

        # ---------------- out = y @ Wp ----------------
        with ExitStack() as dctx:
            outp = dctx.enter_context(tc.tile_pool(name="outp", bufs=4))
            psp_p = dctx.enter_context(tc.tile_pool(name="psp", bufs=3, space="PSUM"))
            for n2 in range(2):
                for sc in range(8):
                    psp = psp_p.tile([P, 512], F32, name="psp", tag="psp")
                    for dc in range(8):
                        mm(psp[:], yt[dc][:, sc * P:(sc + 1) * P],
                           wps[dc][:, n2 * 512:(n2 + 1) * 512], dc == 0, dc == 7)
                    ot = outp.tile([P, 512], F32, name="ot", tag="ot")
                    nc.scalar.copy(ot[:], psp[:])
                    nc.sync.dma_start(
                        out_d[sc * P:(sc + 1) * P, n2 * 512:(n2 + 1) * 512], ot[:])
    nc.compile()
    return nc


def _prep(inputs):
    bf = ml_dtypes.bfloat16
    w_qkv = np.asarray(inputs["w_qkv"], np.float32)
    w_proj = np.asarray(inputs["w_proj"], np.float32)
    cos = np.asarray(inputs["cos"], np.float32).reshape(S, HS // 2)
    sin = np.asarray(inputs["sin"], np.float32).reshape(S, HS // 2)
    wq, wk, wv = w_qkv[:, 0:D], w_qkv[:, D:2 * D], w_qkv[:, 2 * D:3 * D]
    perm = np.empty(D, np.int64)
    for h in range(H):
        b0 = h * HS
        perm[b0:b0 + HS // 2] = b0 + np.arange(0, HS, 2)
        perm[b0 + HS // 2:b0 + HS] = b0 + np.arange(1, HS, 2)
    wq, wk = wq[:, perm], wk[:, perm]
    cosT = np.ascontiguousarray(cos.T)  # [32, S]
    sinT = np.ascontiguousarray(sin.T)
    c1 = np.concatenate([cosT, cosT, cosT, cosT], 0)  # [128, S]
    c2 = np.concatenate([-sinT, sinT, -sinT, sinT], 0)
    mask = np.triu(np.ones((P, P), np.float32))  # [k, q]: allow q >= k
    common = {
        "wq": np.ascontiguousarray(wq).astype(bf),
        "wk": np.ascontiguousarray(wk).astype(bf),
        "wv": np.ascontiguousarray(wv).astype(bf),
        "wp": np.ascontiguousarray(w_proj).astype(bf),
        "c1": c1.astype(bf), "c2": c2.astype(bf), "mask": mask.astype(bf),
        "ones": np.ones((P, H), np.float32).astype(bf),
    }
    return common


LAST_RESULT = None


def kernel(**inputs):
    global LAST_RESULT
    if "nc" not in _CACHE:
        _CACHE["nc"] = _build_nc()
    nc = _CACHE["nc"]
    common = _prep(inputs)
    bf = ml_dtypes.bfloat16
    x = np.asarray(inputs["x"], np.float32)
    in_maps = [
        dict(common, xT=np.ascontiguousarray(x[b].T).astype(bf))
        for b in range(B)
    ]
    res = run_bass_kernel_spmd(nc, in_maps, list(range(NCORES)))
    LAST_RESULT = res
    out = np.stack([res.results[i]["out"] for i in range(B)], 0)
    return out.astype(np.float32)


# revision 26
# speedup vs baseline: 1.8500x; 1.0020x over previous
"""Causal self-attention (RoPE, 16 heads) Trainium2 Bass kernel.

Problem: B=8, S=1024, D=1024, H=16, HS=64, fp32 in/out, causal + all-ones mask.

Strategy: data-parallel over batch — one batch element per NeuronCore (8 cores).
All matmuls in bf16 (fp32 PSUM accumulation); fp32 only for PSUM, softmax
reciprocal, and the final output.

Per-core layout ("transposed activations", no on-chip transposes at all —
x^T is produced on the host):

  x^T   [D, S] bf16  host-transposed, DMA'd straight into SBUF
  Q^T,K^T [D, S]     = W^T @ x^T (lhsT = W tiles), RoPE applied via
                       deinterleaved-head column permutation of W (host) +
                       cos/sin coefficient tiles; swap-halves via DVE
                       partition-offset copies (off the ACT engine)
  V     [S, D]       = x @ W_v, stored per-head with an appended ones column
                       so att@v also yields the softmax denominators
  S^T   [k, q]       = K^T-chunks @ Q^T per head, causal blocks only; the
                       1/sqrt(hs) scale is folded into exp's scale immediate
  att   bf16         = exp(S^T) (no max subtraction: |scores| is small);
                       diagonal blocks masked by a 0/1 triangle multiply
  y^T   [D, S]       accumulated per head; row 64 = denominators; normalize
                       with reciprocal + gpsimd partition broadcast, fused
                       into the PSUM->SBUF move
  out   [S, D] fp32  = y @ W_proj

Emission order interleaves the attention of head-pair fc with the QKV
projection of fc+1 so the PE never idles long enough for the HAM clock gate
to re-throttle it; a burst of warmup matmuls at t=0 lifts the gate while the
initial DMAs stream in.
"""

import os

# The Bass kernel executes through the axon PJRT backend and needs the
# NeuronCores visible; a JAX_PLATFORMS=cpu pin (used for jax reference
# computation) would hide them.
if "axon" not in os.environ.get("JAX_PLATFORMS", "axon"):
    os.environ.pop("JAX_PLATFORMS", None)

import numpy as np
import ml_dtypes
from contextlib import ExitStack

import concourse.bass as bass
import concourse.mybir as mybir
import concourse.tile as tile
from concourse import bacc
from concourse.bass_utils import run_bass_kernel_spmd

B, S, D, H, HS = 8, 1024, 1024, 16, 64
P = 128
NCORES = 8
F32 = mybir.dt.float32
BF16 = mybir.dt.bfloat16
EXP = mybir.ActivationFunctionType.Exp
SCALE = 0.125  # 1/sqrt(HS)

_CACHE = {}
DEBUG = os.environ.get("KDBG", "0") == "1"


def _build_nc():
    nc = bacc.Bacc(
        "TRN2", target_bir_lowering=False, debug=False, num_devices=NCORES)
    xT_d = nc.dram_tensor("xT", [D, S], BF16, kind="ExternalInput")
    wq_d = nc.dram_tensor("wq", [D, D], BF16, kind="ExternalInput")
    wk_d = nc.dram_tensor("wk", [D, D], BF16, kind="ExternalInput")
    wv_d = nc.dram_tensor("wv", [D, D], BF16, kind="ExternalInput")
    wp_d = nc.dram_tensor("wp", [D, D], BF16, kind="ExternalInput")
    c1_d = nc.dram_tensor("c1", [P, S], BF16, kind="ExternalInput")
    c2_d = nc.dram_tensor("c2", [P, S], BF16, kind="ExternalInput")
    mask_d = nc.dram_tensor("mask", [P, P], BF16, kind="ExternalInput")
    ones_d = nc.dram_tensor("ones", [P, H], BF16, kind="ExternalInput")
    out_d = nc.dram_tensor("out", [S, D], F32, kind="ExternalOutput")
    if DEBUG:
        dbgq_d = nc.dram_tensor("dbgq", [D, S], BF16, kind="ExternalOutput")
        dbgk_d = nc.dram_tensor("dbgk", [D, S], BF16, kind="ExternalOutput")
        dbgy_d = nc.dram_tensor("dbgy", [D, S], BF16, kind="ExternalOutput")
        dbgv_d = nc.dram_tensor("dbgv", [S, H * (HS + 1)], BF16,
                                kind="ExternalOutput")

    def mm(out, lhsT, rhs, start, stop):
        nc.tensor.matmul(out, lhsT, rhs, start=start, stop=stop)

    with tile.TileContext(nc) as tc, ExitStack() as ctx:
        persist = ctx.enter_context(tc.tile_pool(name="persist", bufs=1))
        xt = [persist.tile([P, S], BF16, name=f"xt{i}", tag=f"xt{i}") for i in range(8)]
        qt = [persist.tile([P, S], BF16, name=f"qt{i}", tag=f"qt{i}") for i in range(8)]
        kt = [persist.tile([P, S], BF16, name=f"kt{i}", tag=f"kt{i}") for i in range(8)]
        yt = [persist.tile([P, S], BF16, name=f"yt{i}", tag=f"yt{i}") for i in range(8)]
        vt = [persist.tile([P, H, HS + 1], BF16, name=f"vt{i}", tag=f"vt{i}")
              for i in range(8)]
        wqs = [persist.tile([P, D], BF16, name=f"wqs{i}", tag=f"wqs{i}") for i in range(8)]
        wks = [persist.tile([P, D], BF16, name=f"wks{i}", tag=f"wks{i}") for i in range(8)]
        wvs = [persist.tile([P, D], BF16, name=f"wvs{i}", tag=f"wvs{i}") for i in range(8)]
        wps = [persist.tile([P, D], BF16, name=f"wps{i}", tag=f"wps{i}") for i in range(8)]
        c1 = persist.tile([P, S], BF16, name="c1_t", tag="c1_t")
        c2 = persist.tile([P, S], BF16, name="c2_t", tag="c2_t")
        maskt = persist.tile([P, P], BF16, name="maskt", tag="maskt")
        ones_t = persist.tile([P, H], BF16, name="ones_t", tag="ones_t")
        scratch = persist.tile([P, P], BF16, name="scratch", tag="scratch")

        # Warmup source must not depend on any DMA.
        nc.vector.memset(scratch[:], 0.0)

        # DMAs in order of first use: x & wv feed V, coeffs feed the first
        # rope, q/k weights feed the projection loop, wp only at the end.
        for i in range(8):
            nc.sync.dma_start(xt[i][:], xT_d[i * P:(i + 1) * P, :])
            nc.sync.dma_start(wvs[i][:], wv_d[i * P:(i + 1) * P, :])
        for t, d_ in ((c1, c1_d), (c2, c2_d), (maskt, mask_d), (ones_t, ones_d)):
            nc.sync.dma_start(t[:], d_[:])
        for i in range(8):
            nc.sync.dma_start(wqs[i][:], wq_d[i * P:(i + 1) * P, :])
        for i in range(8):
            nc.sync.dma_start(wks[i][:], wk_d[i * P:(i + 1) * P, :])
        for i in range(8):
            nc.sync.dma_start(wps[i][:], wp_d[i * P:(i + 1) * P, :])

        with ExitStack() as mctx:
            pacc = mctx.enter_context(tc.tile_pool(name="pacc", bufs=2, space="PSUM"))
            pss_p = mctx.enter_context(tc.tile_pool(name="pss", bufs=2, space="PSUM"))
            psy_p = mctx.enter_context(tc.tile_pool(name="psy", bufs=1, space="PSUM"))
            attp = mctx.enter_context(tc.tile_pool(name="attp", bufs=9))
            rtmp = mctx.enter_context(tc.tile_pool(name="rtmp", bufs=3))
            smallp = mctx.enter_context(tc.tile_pool(name="smallp", bufs=2))

            # ---- PE warmup: lift the HAM clock gate while DMAs stream ----
            pw = pacc.tile([P, 512], F32, name="pw", tag="acc")
            for _ in range(64):
                nc.tensor.matmul(pw[:, 0:P], scratch[:], scratch[:], start=True,
                                 stop=True)

            # ---------------- V = x @ Wv ----------------
            for f2 in range(2):
                for sc in range(8):
                    ps = pacc.tile([P, 512], F32, name="vps", tag="acc")
                    for dc in range(8):
                        mm(ps[:], xt[dc][:, sc * P:(sc + 1) * P],
                           wvs[dc][:, f2 * 512:(f2 + 1) * 512], dc == 0, dc == 7)
                    nc.vector.tensor_copy(
                        vt[sc][:, f2 * 8:(f2 + 1) * 8, 0:HS],
                        ps[:].rearrange("p (h e) -> p h e", e=HS))
            for sc in range(8):
                nc.vector.tensor_copy(vt[sc][:, :, HS], ones_t[:])

            # ---------------- Q/K projection + RoPE ----------------
            def rope(ps, dst_slice, cc1, cc2, s0):
                # dst = ps * c1 + swap32(ps) * c2.  The swap-halves shuffle is
                # done by the (otherwise idle) DMA engines SBUF->SBUF; the
                # PSUM bank is released after the single ACT copy.
                raw = rtmp.tile([P, 512], BF16, name="ropraw", tag="raw")
                with tc.high_priority(offset=200):
                    if s0 == 0:
                        nc.scalar.copy(raw[:], ps[:])
                    else:
                        nc.vector.tensor_copy(raw[:], ps[:])
                t = rtmp.tile([P, 512], BF16, name="ropet", tag="rt")
                for g, src in ((0, 32), (1, 0), (2, 96), (3, 64)):
                    nc.sync.dma_start(t[g * 32:(g + 1) * 32, :],
                                      raw[src:src + 32, :])
                u = rtmp.tile([P, 512], BF16, name="ropeu", tag="ru")
                nc.vector.tensor_mul(u[:], raw[:], cc1[:, s0:s0 + 512])
                nc.vector.tensor_mul(t[:], t[:], cc2[:, s0:s0 + 512])
                nc.vector.tensor_add(dst_slice, u[:], t[:])

            def qk_half(fc, which):
                wst, dst = (wqs, qt) if which == "q" else (wks, kt)
                for s2 in range(2):
                    ps = pacc.tile([P, 512], F32, name="qkps", tag="acc")
                    for dc in range(8):
                        mm(ps[:], wst[dc][:, fc * P:(fc + 1) * P],
                           xt[dc][:, s2 * 512:(s2 + 1) * 512], dc == 0, dc == 7)
                    rope(ps, dst[fc][:, s2 * 512:(s2 + 1) * 512], c1, c2,
                         s2 * 512)

            # ---------------- attention for head pair ft ----------------
            # Both heads (hb=0/64) of a kc block share one 2-bank PSUM tile:
            # one exp, one sums row, one reciprocal+broadcast per pair.
            def score_pair(ft, qc, kc):
                pss = pss_p.tile([P, 1024], F32, name="pss", tag="pss")
                for hb in (0, 64):
                    nc.tensor.matmul(
                        pss[:, hb * 8:hb * 8 + 512],
                        kt[ft][hb:hb + 64, kc * P:(kc + 1) * P],
                        qt[ft][hb:hb + 64, qc * 512:(qc + 1) * 512],
                        start=True, stop=True, skip_group_check=True)
                att = attp.tile([P, 1024], BF16, name="att", tag="att")
                qsub = kc * P - qc * 512
                if 0 <= qsub < 512:
                    nc.scalar.activation(att[:, qsub:], pss[:, qsub:],
                                         EXP, scale=SCALE)
                    dpair = att[:].rearrange(
                        "p (two c) -> p two c", two=2)[:, :, qsub:qsub + P]
                    nc.vector.tensor_mul(
                        dpair, dpair,
                        maskt[:].unsqueeze(1).broadcast_to([P, 2, P]))
                    return (att, qsub)
                nc.scalar.activation(att[:], pss[:], EXP, scale=SCALE)
                return (att, 0)

            def attv_part(ft, qc, atts):
                # ascending kc: the causally-valid span only narrows, so every
                # accumulation lands inside the start=True footprint and the
                # never-written columns are never read.
                kmax = 4 if qc == 0 else 8
                psy = psy_p.tile([HS + 1, 1024], F32, name="psy", tag="psy")
                for kc in range(kmax):
                    att, lo = atts[kc]
                    nc.tensor.matmul(
                        psy[:, lo:512], vt[kc][:, 2 * ft, :], att[:, lo:512],
                        start=kc == 0, stop=kc == kmax - 1,
                        skip_group_check=True)
                    nc.tensor.matmul(
                        psy[:, 512 + lo:], vt[kc][:, 2 * ft + 1, :],
                        att[:, 512 + lo:],
                        start=kc == 0, stop=kc == kmax - 1,
                        skip_group_check=True)
                srow = smallp.tile([1, 1024], F32, name="srow", tag="rr")
                nc.vector.tensor_copy(srow[:], psy[HS:HS + 1, :])
                nc.vector.reciprocal_approx_fast(out=srow[:], in_=srow[:])
                rb = smallp.tile([P, 1024], F32, name="rb", tag="rb")
                nc.gpsimd.partition_broadcast(rb[:], srow[0:1, :])
                nc.vector.tensor_mul(
                    yt[ft][0:64, qc * 512:(qc + 1) * 512],
                    psy[0:HS, 0:512], rb[0:64, 0:512])
                nc.vector.tensor_mul(
                    yt[ft][64:128, qc * 512:(qc + 1) * 512],
                    psy[0:HS, 512:1024], rb[64:128, 512:1024])

            def qk_mms(fc, which):
                # yields the 16 projection matmuls + the 2 rope tails as
                # closures, so the caller can interleave them with score work.
                wst, dst = (wqs, qt) if which == "q" else (wks, kt)
                units = []
                pss_tiles = {}

                def proj_mm(s2, dc):
                    def emit():
                        if dc == 0:
                            pss_tiles[s2] = pacc.tile(
                                [P, 512], F32, name="qkps", tag="acc")
                        nc.tensor.matmul(
                            pss_tiles[s2][:],
                            wst[dc][:, fc * P:(fc + 1) * P],
                            xt[dc][:, s2 * 512:(s2 + 1) * 512],
                            start=dc == 0, stop=dc == 7,
                            skip_group_check=True)
                        if dc == 7:
                            rope(pss_tiles[s2],
                                 dst[fc][:, s2 * 512:(s2 + 1) * 512],
                                 c1, c2, s2 * 512)
                    return emit

                for s2 in range(2):
                    for dc in range(8):
                        units.append(proj_mm(s2, dc))
                return units

            def run_units(units, n):
                for _ in range(n):
                    if units:
                        units.pop(0)()

            def dummy_mms(n):
                # keep the HAM clock gate open through the ACT-paced tail of
                # the last head pair, so the projection starts at full clock
                units = []

                def emit():
                    pw2 = pacc.tile([P, 512], F32, name="pw2", tag="acc")
                    nc.tensor.matmul(pw2[:], scratch[:], xt[0][:, 0:512],
                                     start=True, stop=True)
                return [emit] * n

            # prologue: Q(0), K(0)
            for u in qk_mms(0, "q"):
                u()
            for u in qk_mms(0, "k"):
                u()
            for fc in range(8):
                qu = qk_mms(fc + 1, "q") if fc < 7 else dummy_mms(8)
                ku = qk_mms(fc + 1, "k") if fc < 7 else dummy_mms(16)
                a0 = []
                for kc in range(4):
                    a0.append(score_pair(fc, 0, kc))
                    run_units(qu, 4)
                run_units(qu, 16)
                attv_part(fc, 0, a0)
                a1 = []
                for kc in range(8):
                    a1.append(score_pair(fc, 1, kc))
                    run_units(ku, 2)
                run_units(ku, 16)
                attv_part(fc, 1, a1)

        if DEBUG:
            for i in range(8):
                nc.sync.dma_start(dbgq_d[i * P:(i + 1) * P, :], qt[i][:])
                nc.sync.dma_start(dbgk_d[i * P:(i + 1) * P, :], kt[i][:])
                nc.sync.dma_start(dbgy_d[i * P:(i + 1) * P, :], yt[i][:])
                nc.sync.dma_start(
                    dbgv_d[i * P:(i + 1) * P, :],
                    vt[i][:].rearrange("p h e -> p (h e)"))

        # ---------------- out = y @ Wp ----------------
        with ExitStack() as dctx:
            outp = dctx.enter_context(tc.tile_pool(name="outp", bufs=4))
            psp_p = dctx.enter_context(tc.tile_pool(name="psp", bufs=3, space="PSUM"))
            for n2 in range(2):
                for sc in range(8):
                    psp = psp_p.tile([P, 512], F32, name="psp", tag="psp")
                    for dc in range(8):
                        mm(psp[:], yt[dc][:, sc * P:(sc + 1) * P],
                           wps[dc][:, n2 * 512:(n2 + 1) * 512], dc == 0, dc == 7)
                    ot = outp.tile([P, 512], F32, name="ot", tag="ot")
                    nc.scalar.copy(ot[:], psp[:])
                    nc.sync.dma_start(
                        out_d[sc * P:(sc + 1) * P, n2 * 512:(n2 + 1) * 512], ot[:])
    nc.compile()
    return nc


def _prep(inputs):
    bf = ml_dtypes.bfloat16
    w_qkv = np.asarray(inputs["w_qkv"], np.float32)
    w_proj = np.asarray(inputs["w_proj"], np.float32)
    cos = np.asarray(inputs["cos"], np.float32).reshape(S, HS // 2)
    sin = np.asarray(inputs["sin"], np.float32).reshape(S, HS // 2)
    wq, wk, wv = w_qkv[:, 0:D], w_qkv[:, D:2 * D], w_qkv[:, 2 * D:3 * D]
    perm = np.empty(D, np.int64)
    for h in range(H):
        b0 = h * HS
        perm[b0:b0 + HS // 2] = b0 + np.arange(0, HS, 2)
        perm[b0 + HS // 2:b0 + HS] = b0 + np.arange(1, HS, 2)
    wq, wk = wq[:, perm], wk[:, perm]
    cosT = np.ascontiguousarray(cos.T)  # [32, S]
    sinT = np.ascontiguousarray(sin.T)
    c1 = np.concatenate([cosT, cosT, cosT, cosT], 0)  # [128, S]
    c2 = np.concatenate([-sinT, sinT, -sinT, sinT], 0)
    mask = np.triu(np.ones((P, P), np.float32))  # [k, q]: allow q >= k
    common = {
        "wq": np.ascontiguousarray(wq).astype(bf),
        "wk": np.ascontiguousarray(wk).astype(bf),
        "wv": np.ascontiguousarray(wv).astype(bf),
        "wp": np.ascontiguousarray(w_proj).astype(bf),
        "c1": c1.astype(bf), "c2": c2.astype(bf), "mask": mask.astype(bf),
        "ones": np.ones((P, H), np.float32).astype(bf),
    }
    return common


LAST_RESULT = None


def kernel(**inputs):
    global LAST_RESULT
    if "nc" not in _CACHE:
        _CACHE["nc"] = _build_nc()
    nc = _CACHE["nc"]
    common = _prep(inputs)
    bf = ml_dtypes.bfloat16
    x = np.asarray(inputs["x"], np.float32)
    in_maps = [
        dict(common, xT=np.ascontiguousarray(x[b].T).astype(bf))
        for b in range(B)
    ]
    res = run_bass_kernel_spmd(nc, in_maps, list(range(NCORES)))
    LAST_RESULT = res
    out = np.stack([res.results[i]["out"] for i in range(B)], 0)
    return out.astype(np.float32)
